# revision 1
# baseline (speedup 1.0000x reference)
"""MemoryEfficientAttention on 8 TRN2 NeuronCores.

Full inputs in, full output out. Sharding: data-parallel over batch (2) x
tensor-parallel over heads (16 heads -> 4 heads/core). Each core computes
qkv projection for its heads, flash-style attention, and a partial output
projection over its 256 head-dims; the host sums the 4 partial projections
per batch and adds the bias.

All matmuls run as float32r (TF32-like, 1 cycle/row at N>=256 vs 4 for
fp32), measured ~2e-4 max rel err per matmul on HW.

Device layouts (T = transposed so the contraction dim is on partitions):
  xT  [1024, 2048]  x[b]^T                      (rhs of q/k, lhsT of v)
  wqT/wkT/wvT [1024, 256]  qkv_w slices^T       (lhsT of q/k, rhs of v)
  pwT [256, 1024]   proj_w column-slice^T       (rhs of proj)
  q^T/k^T computed as [d, n] (head-dim on partitions) so S^T = k^T-block
  matmuls need no transposes; V computed as [n, d]; PV matmul folds the
  softmax denominator via a ones column appended to V (Z lands on psum
  partition 64); normalization = reciprocal + gpsimd partition_broadcast +
  one DVE multiply, applied before the output projection.
"""

import numpy as np

B, N, C = 2, 2048, 1024
H, HD = 16, 64
NCORES = 8
TPG = 4              # tensor-parallel cores per batch
HPC = H // TPG       # 4 heads per core
D = HPC * HD         # 256 local head dims
KO = C // 128        # 8 contraction subtiles of the model dim
NB = N // 128        # 16 token blocks
MB = N // 128        # 16 key blocks
NT = 1024            # query-tile width in attention
NTC = N // NT
SCALE = HD ** -0.5

_state = {}


def _build_nc(reps=1, phase="full", dtype="f32r", opts=None):
    import concourse.bass as bass
    import concourse.tile as tile
    import concourse.mybir as mybir
    from concourse import bacc

    opts = {**dict(ps_s_bufs=3, ps_o_bufs=1, mm_bufs=1, eb_bufs=2,
                   outp_bufs=2, big_y=True, xt_rows=True, nt=1024,
                   early_free=True, v_first=True, pe_bcast=False,
                   pair_heads=False, mm_share=True),
            **(opts or {})}
    NT = opts["nt"]
    NTC = N // NT
    f32 = mybir.dt.float32
    f32r = mybir.dt.float32r
    mdt = f32r if dtype == "f32r" else mybir.dt.bfloat16
    Exp = mybir.ActivationFunctionType.Exp
    mult = mybir.AluOpType.mult

    nc = bacc.Bacc("TRN2", target_bir_lowering=False, debug=False,
                   num_devices=NCORES)

    xT_d = nc.dram_tensor("xT", [C, N], mdt, kind="ExternalInput")
    wqT_d = nc.dram_tensor("wqT", [C, D], mdt, kind="ExternalInput")
    wkT_d = nc.dram_tensor("wkT", [C, D], mdt, kind="ExternalInput")
    wvT_d = nc.dram_tensor("wvT", [C, D], mdt, kind="ExternalInput")
    pwT_d = nc.dram_tensor("pwT", [D, C], mdt, kind="ExternalInput")
    ones_d = nc.dram_tensor("ones", [NB * HPC], mdt, kind="ExternalInput")
    y_d = nc.dram_tensor("y", [N, C], f32, kind="ExternalOutput")

    with tile.TileContext(nc) as tc:
        with (
            tc.tile_pool(name="big", bufs=1) as big,
            tc.tile_pool(name="work", bufs=2) as work,
            tc.tile_pool(name="ebp", bufs=opts["eb_bufs"]) as ebp,
            tc.tile_pool(name="outp", bufs=opts["outp_bufs"]) as outp,
            tc.tile_pool(name="ps_mm", bufs=opts["mm_bufs"], space="PSUM") as ps_mm,
            tc.tile_pool(name="ps_s", bufs=opts["ps_s_bufs"], space="PSUM") as ps_s,
            tc.tile_pool(name="ps_o", bufs=opts["ps_o_bufs"], space="PSUM") as ps_o,
        ):
            xt = big.tile([128, KO, N], mdt, tag="xt")
            wq = big.tile([128, KO, D], mdt, tag="wq")
            wk = big.tile([128, KO, D], mdt, tag="wk")
            wv = big.tile([128, KO, D], mdt, tag="wv")
            pw = big.tile([128, D // 128, C], mdt, tag="pw")
            qt = [big.tile([128, N], mdt, tag=f"qt{t}", name=f"qt{t}")
                  for t in range(2)]
            kt = [big.tile([128, N], mdt, tag=f"kt{t}", name=f"kt{t}")
                  for t in range(2)]
            vt = big.tile([128, NB, HPC * (HD + 1)], mdt, tag="vt")
            ot = [big.tile([128, N], mdt, tag=f"ot{t}", name=f"ot{t}")
                  for t in range(2)]
            vt4 = vt[:].rearrange("p nb (h c) -> p nb h c", c=HD + 1)
            ebc = (big.tile([128, NT], mdt, tag="ebc", name="ebc")
                   if phase == "attn_noexp" else None)

            def emit_body():
                # ---- loads ----
                nc.sync.dma_start(
                    wq[:], wqT_d.ap().rearrange("(ko p) d -> p ko d", p=128))
                nc.sync.dma_start(
                    wk[:], wkT_d.ap().rearrange("(ko p) d -> p ko d", p=128))
                nc.sync.dma_start(
                    wv[:], wvT_d.ap().rearrange("(ko p) d -> p ko d", p=128))
                nc.sync.dma_start(
                    pw[:], pwT_d.ap().rearrange("(t p) e -> p t e", p=128))
                if opts["xt_rows"]:
                    # split each ko row-block into token halves so the first
                    # qk/V matmuls start after half the 8MB load
                    for hf in range(2):
                        hsl = slice(hf * (N // 2), (hf + 1) * (N // 2))
                        for ko in range(KO):
                            nc.sync.dma_start(
                                xt[:, ko, hsl],
                                xT_d.ap()[ko * 128:(ko + 1) * 128, hsl])
                else:
                    for ch in range(4):
                        s = slice(ch * 512, (ch + 1) * 512)
                        nc.sync.dma_start(
                            xt[:, :, s],
                            xT_d.ap()[:, s].rearrange("(ko p) n -> p ko n", p=128))
                # ones column of vt: memset f32 staging + DVE cast-copy
                ones_sb = work.tile([128, NB * HPC], f32, tag="ones_sb",
                                    name="ones_sb", bufs=1)
                nc.vector.memset(ones_sb[:], 1.0)
                nc.vector.tensor_copy(
                    vt4[:, :, :, HD:HD + 1],
                    ones_sb[:].rearrange("p (nb h) -> p nb h", nb=NB
                                         ).unsqueeze(-1))

                # ---- qkv projection ----
                def emit_v():
                    # V in [n, d] layout: lhsT = xT block, rhs = wv
                    for nb in range(NB):
                        bsl = slice(nb * 128, (nb + 1) * 128)
                        mmp = ps_s if opts["mm_share"] else ps_mm
                        mmt = "ps" if opts["mm_share"] else "mm"
                        pm = mmp.tile([128, 512], f32, tag=mmt, name="pm")
                        for ko in range(KO):
                            nc.tensor.matmul(
                                pm[:, :D], xt[:, ko, bsl], wv[:, ko, :],
                                start=(ko == 0), stop=(ko == KO - 1))
                        nc.vector.tensor_copy(
                            vt4[:, nb, :, 0:HD],
                            pm[:, :D].rearrange("p (h c) -> p h c", c=HD))

                def emit_qk(order):
                    # q^T / k^T in [d, n] layout: lhsT = w slice, rhs = xT
                    for w, dst, t in order:
                        dsl = slice(t * 128, (t + 1) * 128)
                        for ch in range(4):
                            nsl = slice(ch * 512, (ch + 1) * 512)
                            mmp = ps_s if opts["mm_share"] else ps_mm
                            mmt = "ps" if opts["mm_share"] else "mm"
                            pm = mmp.tile([128, 512], f32, tag=mmt,
                                          name="pm")
                            for ko in range(KO):
                                nc.tensor.matmul(
                                    pm[:], w[:, ko, dsl], xt[:, ko, nsl],
                                    start=(ko == 0), stop=(ko == KO - 1))
                            nc.vector.tensor_copy(dst[t][:, nsl], pm[:])

                if opts["v_first"]:
                    emit_v()
                    emit_qk([(wq, qt, 0), (wk, kt, 0),
                             (wq, qt, 1), (wk, kt, 1)])
                else:
                    emit_qk([(wq, qt, 0), (wq, qt, 1),
                             (wk, kt, 0), (wk, kt, 1)])
                    emit_v()

                if phase == "qkv":
                    # dump q/k/v so nothing is dead-code-eliminated
                    yf = y_d.ap().rearrange("n c -> (n c)")
                    ofs = 0
                    for tl in (qt[0], qt[1], kt[0], kt[1]):
                        sz = 128 * N
                        nc.sync.dma_start(
                            yf[ofs:ofs + sz].rearrange("(p f) -> p f", p=128),
                            tl[:].bitcast(f32))
                        ofs += sz
                    sz = 128 * NB * HPC * (HD + 1)
                    nc.sync.dma_start(
                        yf[ofs:ofs + sz].rearrange("(p f) -> p f", p=128),
                        vt[:].bitcast(f32))
                    return

                # ---- attention + projection ----
                if phase == "attn_noexp":
                    nc.scalar.activation(
                        out=ebc[:], in_=qt[0][:, 0:NT], func=Exp, scale=SCALE)
                for nt in range(NTC):
                    qsl = slice(nt * NT, (nt + 1) * NT)
                    for h in range(HPC):
                        t, hi = divmod(h, 2)
                        psl = slice(hi * 64, (hi + 1) * 64)
                        po = ps_o.tile([HD + 1, NT], f32, tag="po", name="po")
                        for mb in range(MB):
                            msl = slice(mb * 128, (mb + 1) * 128)
                            psb = ps_s.tile([128, NT], f32, tag="ps",
                                            name="psb")
                            for sc in range(NT // 512):
                                ssl = slice(sc * 512, (sc + 1) * 512)
                                nc.tensor.matmul(
                                    psb[:, ssl],
                                    kt[t][psl, msl],
                                    qt[t][psl, nt * NT + sc * 512:
                                          nt * NT + (sc + 1) * 512],
                                    start=True, stop=True)
                            if phase == "attn_noexp":
                                eb = ebc
                            else:
                                eb = ebp.tile([128, NT], mdt, tag="eb",
                                              name="eb")
                                nc.scalar.activation(
                                    out=eb[:], in_=psb[:], func=Exp,
                                    scale=SCALE)
                            for sc in range(NT // 512):
                                ssl = slice(sc * 512, (sc + 1) * 512)
                                nc.tensor.matmul(
                                    po[:, ssl],
                                    vt4[:, mb, h, :],
                                    eb[:, ssl],
                                    start=(mb == 0), stop=(mb == MB - 1))
                        # normalize: O^T[dh, n] * (1/Z[n])
                        rz = work.tile([1, NT], f32, tag="rz", name="rz")
                        if opts["early_free"]:
                            # one copy frees po for the next group; the
                            # 3-hop normalize chain then runs off-path
                            poc = work.tile([HD + 1, NT], f32, tag="poc",
                                            name="poc", bufs=1)
                            nc.vector.tensor_copy(poc[:], po[:])
                            src_o = poc[0:HD, :]
                            nc.vector.reciprocal(rz[:], poc[HD:HD + 1, :])
                        else:
                            src_o = po[0:HD, :]
                            nc.vector.reciprocal(rz[:], po[HD:HD + 1, :])
                        if opts["pe_bcast"]:
                            # broadcast 1/Z across partitions via K=1 matmul
                            rzp = ps_mm.tile([128, 512], f32, tag="mm",
                                             name="rzp")
                            for sc in range(NT // 512):
                                nc.tensor.matmul(
                                    rzp[0:64, 0:512],
                                    ones_sb[0:1, 0:64],
                                    rz[:, sc * 512:(sc + 1) * 512],
                                    start=True, stop=True)
                                # copy to sbuf half (DVE)
                                if sc == 0:
                                    rzb = work.tile([64, NT], f32, tag="rzb",
                                                    name="rzb")
                                nc.vector.tensor_copy(
                                    rzb[:, sc * 512:(sc + 1) * 512],
                                    rzp[0:64, 0:512])
                        else:
                            rzb = work.tile([64, NT], f32, tag="rzb",
                                            name="rzb")
                            nc.gpsimd.partition_broadcast(rzb[:], rz[:])
                        nc.vector.tensor_tensor(
                            ot[t][psl, qsl], src_o, rzb[:], mult)

                    if phase in ("attn", "attn_noexp"):
                        continue
                    # partial output projection for this query tile
                    for nb in range(nt * NT // 128, (nt + 1) * NT // 128):
                        bsl = slice(nb * 128, (nb + 1) * 128)
                        ybig = (outp.tile([128, C], f32, tag="ybig",
                                          name="ybig")
                                if opts["big_y"] else None)
                        for ech in range(2):
                            esl = slice(ech * 512, (ech + 1) * 512)
                            mmp = ps_s if opts["mm_share"] else ps_mm
                            mmt = "ps" if opts["mm_share"] else "mm"
                            py = mmp.tile([128, 512], f32, tag=mmt,
                                          name="py")
                            for t in range(2):
                                nc.tensor.matmul(
                                    py[:], ot[t][:, bsl], pw[:, t, esl],
                                    start=(t == 0), stop=(t == 1))
                            if opts["big_y"]:
                                nc.vector.tensor_copy(ybig[:, esl], py[:])
                            else:
                                yb = outp.tile([128, 512], f32, tag="yb",
                                               name="yb")
                                nc.vector.tensor_copy(yb[:], py[:])
                                nc.sync.dma_start(y_d.ap()[bsl, esl], yb[:])
                        if opts["big_y"]:
                            nc.sync.dma_start(y_d.ap()[bsl, :], ybig[:])

                if phase in ("attn", "attn_noexp"):
                    yf = y_d.ap().rearrange("n c -> (n c)")
                    for i, tl in enumerate(ot):
                        sz = 128 * N
                        nc.sync.dma_start(
                            yf[i * sz:(i + 1) * sz].rearrange(
                                "(p f) -> p f", p=128),
                            tl[:].bitcast(f32))

            if reps == 1:
                emit_body()
            else:
                # device-side hardware loop: one dispatch, reps executions
                with tc.For_i(0, reps, 1):
                    emit_body()

    nc.compile()
    return nc


def _get_nc(reps=1, phase="full", dtype="f32r", opts=None):
    key = f"nc{reps}-{phase}-{dtype}-{sorted((opts or {}).items())}"
    if key not in _state:
        _state[key] = _build_nc(reps, phase, dtype, opts)
    return _state[key]


def _shard_inputs(x, qkv_w, proj_w, dtype="f32r"):
    """Per-core input maps. Core c: batch c//4, heads 4*(c%4)..4*(c%4)+3."""
    if dtype == "f32r":
        cast = lambda a: np.ascontiguousarray(a, np.float32)
    else:
        import ml_dtypes
        cast = lambda a: np.ascontiguousarray(a).astype(ml_dtypes.bfloat16)
    ones = cast(np.ones(NB * HPC, np.float32))
    in_maps = []
    for c in range(NCORES):
        b, g = divmod(c, TPG)
        dsl = slice(g * D, (g + 1) * D)
        in_maps.append({
            "xT": cast(x[b].T),
            "wqT": cast(qkv_w[dsl, :].T),
            "wkT": cast(qkv_w[C:2 * C][dsl, :].T),
            "wvT": cast(qkv_w[2 * C:][dsl, :].T),
            "pwT": cast(proj_w[:, dsl].T),
            "ones": ones,
        })
    return in_maps


def _make_runner(nc, donate=True):
    """Jitted 8-core SPMD runner for a built Bass module."""
    import jax
    import concourse.mybir as mybir
    from concourse import bass2jax

    bass2jax.install_neuronx_cc_hook()

    partition_name = (nc.partition_id_tensor.name
                      if nc.partition_id_tensor else None)
    in_names, out_names, out_avals, zero_shapes = [], [], [], []
    for alloc in nc.m.functions[0].allocations:
        if not isinstance(alloc, mybir.MemoryLocationSet):
            continue
        name = alloc.memorylocations[0].name
        if alloc.kind == "ExternalInput":
            if name != partition_name:
                in_names.append(name)
        elif alloc.kind == "ExternalOutput":
            shape = tuple(alloc.tensor_shape)
            dtype = mybir.dt.np(alloc.dtype)
            out_names.append(name)
            out_avals.append(jax.core.ShapedArray(shape, dtype))
            zero_shapes.append((shape, dtype))
    n_params = len(in_names)
    all_in_names = list(in_names) + list(out_names)
    if partition_name is not None:
        all_in_names.append(partition_name)
    donate_idx = tuple(range(n_params, n_params + len(out_names))) if donate \
        else ()

    def _body(*args):
        operands = list(args)
        if partition_name is not None:
            operands.append(bass2jax.partition_id_tensor())
        outs = bass2jax._bass_exec_p.bind(
            *operands,
            out_avals=tuple(out_avals),
            in_names=tuple(all_in_names),
            out_names=tuple(out_names),
            lowering_input_output_aliases=(),
            sim_require_finite=True,
            sim_require_nnan=True,
            nc=nc,
        )
        return tuple(outs)

    devices = jax.devices()[:NCORES]
    mesh = bass2jax.Mesh(np.asarray(devices), ("core",))
    spec = (bass2jax.PartitionSpec("core"),)
    sharded = jax.jit(
        bass2jax.shard_map(
            _body, mesh=mesh,
            in_specs=spec * (n_params + len(out_names)),
            out_specs=spec * len(out_names),
            check_rep=False),
        donate_argnums=donate_idx, keep_unused=True)

    meta = dict(in_names=in_names, out_names=out_names, out_avals=out_avals,
                zero_shapes=zero_shapes, mesh=mesh)
    return sharded, meta


def _get_runner():
    if "runner" in _state:
        return _state["runner"]
    nc = _get_nc(1)
    sharded, meta = _make_runner(nc, donate=True)

    def run(in_maps):
        concat_in = [
            np.concatenate([np.asarray(m[name]) for m in in_maps], axis=0)
            for name in meta["in_names"]
        ]
        concat_zeros = [
            np.zeros((NCORES * s[0], *s[1:]), dt)
            for s, dt in meta["zero_shapes"]
        ]
        out_arrs = sharded(*concat_in, *concat_zeros)
        out_avals = meta["out_avals"]
        return [
            {name: np.asarray(out_arrs[i]).reshape(
                NCORES, *out_avals[i].shape)[c]
             for i, name in enumerate(meta["out_names"])}
            for c in range(NCORES)
        ]

    _state["runner"] = run
    return run


def _combine(results, proj_b):
    """Sum the 4 tensor-parallel partial projections per batch, add bias."""
    out = np.empty((B, N, C), np.float32)
    for b in range(B):
        acc = results[b * TPG + 0]["y"].astype(np.float32).copy()
        for g in range(1, TPG):
            acc += results[b * TPG + g]["y"]
        out[b] = acc + proj_b[None, :]
    return out


def kernel(x, qkv_w, proj_w, proj_b):
    x = np.asarray(x, np.float32)
    qkv_w = np.asarray(qkv_w, np.float32)
    proj_w = np.asarray(proj_w, np.float32)
    proj_b = np.asarray(proj_b, np.float32)
    run = _get_runner()
    results = run(_shard_inputs(x, qkv_w, proj_w))
    return _combine(results, proj_b)


def make_timing_fn(reps, in_maps, phase="full", dtype="f32r", opts=None):
    """Device-resident, non-donating executor of the reps-times kernel.

    Returns fn() that launches one execution and blocks until done. Inputs
    (and dummy zero outputs) are placed on device once, so repeated calls
    measure dispatch + on-device execution only.
    """
    import jax
    from jax.sharding import NamedSharding
    from concourse import bass2jax

    nc = _get_nc(reps, phase, dtype, opts)
    sharded, meta = _make_runner(nc, donate=False)
    shd = NamedSharding(meta["mesh"], bass2jax.PartitionSpec("core"))
    dev_in = [
        jax.device_put(
            np.concatenate([np.asarray(m[name]) for m in in_maps], axis=0),
            shd)
        for name in meta["in_names"]
    ]
    dev_zero = [
        jax.device_put(np.zeros((NCORES * s[0], *s[1:]), dt), shd)
        for s, dt in meta["zero_shapes"]
    ]

    def fn():
        outs = sharded(*dev_in, *dev_zero)
        for o in outs:
            o.block_until_ready()
        return outs

    return fn



# revision 5
# speedup vs baseline: 1.1766x; 1.1766x over previous
"""MemoryEfficientAttention on 8 TRN2 NeuronCores.

Full inputs in, full output out. Sharding: data-parallel over batch (2) x
tensor-parallel over heads (16 heads -> 4 heads/core). Each core computes
qkv projection for its heads, flash-style attention, and a partial output
projection over its 256 head-dims; the host sums the 4 partial projections
per batch and adds the bias.

V2: all operands bf16 (rel err ~7e-3 vs 2e-2 budget), f32 psum accumulate.
The kernel is emitted as a software-pipelined stream built around the Act
engine (exp is the densest fixed cost: 16.8M exps = ~110us at 128 lanes *
1.2GHz). A virtual-clock scheduler interleaves background PE work (qkv
chunk matmuls, V blocks, output-projection blocks) into the attention
S->exp->PV stream so exp starts ~10us in and never starves:
  upfront: q0c0,q0c1,k0c0,V0; then 128 steps of (S, exp, PV) per
  (head, query-tile, key-block) with deadline-driven drain of queued units.
Device layouts (T = contraction dim on partitions):
  xt  [128,8ko,2048]  x[b]^T          qt/kt [128,2048] per head-pair
  vt  [128,16nb,4h,65] V with a ones column (softmax Z via PV matmul)
  po  [65,1024] psum   S^T [128keys,1024q] psum -> exp -> eb bf16
  normalization: reciprocal + gpsimd partition_broadcast + DVE multiply.
DMA: inputs on sync queue (weights first, xt in 4 token-chunks so the
first matmuls start after ~3us); y stores on the gpsimd queue.
"""

import heapq
import numpy as np

B, N, C = 2, 2048, 1024
H, HD = 16, 64
NCORES = 8
TPG = 4              # tensor-parallel cores per batch
HPC = H // TPG       # 4 heads per core
D = HPC * HD         # 256 local head dims
KO = C // 128        # 8 contraction subtiles of the model dim
NB = N // 128        # 16 token blocks
MB = N // 128        # 16 key blocks
NT = 1024            # query-tile width in attention
NTC = N // NT
SCALE = HD ** -0.5

_state = {}


def _build_nc(reps=1, phase="full", dtype="bf16", opts=None):
    import concourse.bass as bass
    import concourse.tile as tile
    import concourse.mybir as mybir
    from concourse import bacc

    opts = {**dict(ps_bufs=2, mm_bufs=2, eb_bufs=4, outp_bufs=2,
                   po_bufs=1, act_ns=1070, mm_ns=284, mm256_ns=178),
            **(opts or {})}
    f32 = mybir.dt.float32
    mdt = mybir.dt.bfloat16 if dtype == "bf16" else mybir.dt.float32r
    Exp = mybir.ActivationFunctionType.Exp
    mult = mybir.AluOpType.mult
    ACT_NS = opts["act_ns"]
    MM_NS = opts["mm_ns"]       # virtual cost of one N=512 matmul
    MM256_NS = opts["mm256_ns"]

    nc = bacc.Bacc("TRN2", target_bir_lowering=False, debug=False,
                   num_devices=NCORES)

    xT_d = nc.dram_tensor("xT", [C, N], mdt, kind="ExternalInput")
    wqT_d = nc.dram_tensor("wqT", [C, D], mdt, kind="ExternalInput")
    wkT_d = nc.dram_tensor("wkT", [C, D], mdt, kind="ExternalInput")
    wvT_d = nc.dram_tensor("wvT", [C, D], mdt, kind="ExternalInput")
    pwT_d = nc.dram_tensor("pwT", [D, C], mdt, kind="ExternalInput")
    y_d = nc.dram_tensor("y", [N, C], f32, kind="ExternalOutput")

    with tile.TileContext(nc) as tc:
        with (
            tc.tile_pool(name="big", bufs=1) as big,
            tc.tile_pool(name="work", bufs=2) as work,
            tc.tile_pool(name="ebp", bufs=opts["eb_bufs"]) as ebp,
            tc.tile_pool(name="outp", bufs=opts["outp_bufs"]) as outp,
            tc.tile_pool(name="ps_mm", bufs=opts["mm_bufs"], space="PSUM") as ps_mm,
            tc.tile_pool(name="ps_s", bufs=opts["ps_bufs"], space="PSUM") as ps_s,
            tc.tile_pool(name="ps_o", bufs=opts["po_bufs"], space="PSUM") as ps_o,
        ):
            xt = big.tile([128, KO, N], mdt, tag="xt")
            wq = big.tile([128, KO, D], mdt, tag="wq")
            wk = big.tile([128, KO, D], mdt, tag="wk")
            wv = big.tile([128, KO, D], mdt, tag="wv")
            pw = big.tile([128, D // 128, C], mdt, tag="pw")
            qt = [big.tile([128, N], mdt, tag=f"qt{t}", name=f"qt{t}")
                  for t in range(2)]
            kt = [big.tile([128, N], mdt, tag=f"kt{t}", name=f"kt{t}")
                  for t in range(2)]
            vt = big.tile([128, NB, HPC * (HD + 1)], mdt, tag="vt")
            ot = [big.tile([128, N], mdt, tag=f"ot{t}", name=f"ot{t}")
                  for t in range(2)]
            vt4 = vt[:].rearrange("p nb (h c) -> p nb h c", c=HD + 1)

            def emit_body():
                # ---- input DMA: weights first, xt in 4 token-chunks ----
                nc.sync.dma_start(
                    wq[:], wqT_d.ap().rearrange("(ko p) d -> p ko d", p=128))
                nc.sync.dma_start(
                    wk[:], wkT_d.ap().rearrange("(ko p) d -> p ko d", p=128))
                nc.sync.dma_start(
                    wv[:], wvT_d.ap().rearrange("(ko p) d -> p ko d", p=128))
                for ch in range(4):
                    s = slice(ch * 512, (ch + 1) * 512)
                    nc.sync.dma_start(
                        xt[:, :, s],
                        xT_d.ap()[:, s].rearrange("(ko p) n -> p ko n", p=128))
                nc.sync.dma_start(
                    pw[:], pwT_d.ap().rearrange("(t p) e -> p t e", p=128))
                # ones column of vt: memset f32 staging + DVE cast-copy
                ones_sb = work.tile([128, NB * HPC], f32, tag="ones_sb",
                                    name="ones_sb", bufs=1)
                nc.vector.memset(ones_sb[:], 1.0)
                nc.vector.tensor_copy(
                    vt4[:, :, :, HD:HD + 1],
                    ones_sb[:].rearrange("p (nb h) -> p nb h", nb=NB
                                         ).unsqueeze(-1))

                # ---- background unit machinery ----
                clock = {"pe": 0.0, "act": 0.0}
                Q = []
                ctr = [0]

                def push(due, fn, cost):
                    ctr[0] += 1
                    heapq.heappush(Q, (due, ctr[0], fn, cost))

                def drain(s, slack=False):
                    while Q and (Q[0][0] <= s
                                 or (slack and clock["pe"] < clock["act"])):
                        _, _, fn, cost = heapq.heappop(Q)
                        fn()
                        clock["pe"] += cost

                def qk_units(w, dst, t, ch):
                    """q^T/k^T chunk: 8 ko-matmuls [128,512] + copy, 4 units."""
                    nsl = slice(ch * 512, (ch + 1) * 512)
                    dsl = slice(t * 128, (t + 1) * 128)
                    st = {}

                    def mk(i):
                        def f():
                            if i == 0:
                                st["pm"] = ps_mm.tile([128, 512], f32,
                                                      tag="mm", name="pm")
                            pm = st["pm"]
                            for ko in (2 * i, 2 * i + 1):
                                nc.tensor.matmul(
                                    pm[:], w[:, ko, dsl], xt[:, ko, nsl],
                                    start=(ko == 0), stop=(ko == KO - 1))
                            if i == 3:
                                nc.vector.tensor_copy(dst[:, nsl], pm[:])
                        return f
                    return [(mk(i), 2 * MM_NS) for i in range(4)]

                def v_units(nb):
                    """V block: 8 ko-matmuls [128,256] + copy, 4 units."""
                    bsl = slice(nb * 128, (nb + 1) * 128)
                    st = {}

                    def mk(i):
                        def f():
                            if i == 0:
                                st["pm"] = ps_mm.tile([128, 512], f32,
                                                      tag="mm", name="pm")
                            pm = st["pm"]
                            for ko in (2 * i, 2 * i + 1):
                                nc.tensor.matmul(
                                    pm[:, :D], xt[:, ko, bsl], wv[:, ko, :],
                                    start=(ko == 0), stop=(ko == KO - 1))
                            if i == 3:
                                nc.vector.tensor_copy(
                                    vt4[:, nb, :, 0:HD],
                                    pm[:, :D].rearrange("p (h c) -> p h c",
                                                        c=HD))
                        return f
                    return [(mk(i), 2 * MM256_NS) for i in range(4)]

                def proj_units(nt, nb):
                    """partial output projection of token block nb: 2 units.

                    y stores: nt0 on the gpsimd queue (Act still busy with
                    exps then), nt1 on the Act queue (idle in the tail, and
                    keeps the last partition_broadcasts unblocked on Pool).
                    """
                    bsl = slice(nb * 128, (nb + 1) * 128)
                    dma_eng = nc.gpsimd if nt == 0 else nc.scalar
                    st = {}

                    def mk(ech):
                        esl = slice(ech * 512, (ech + 1) * 512)

                        def f():
                            if ech == 0:
                                st["yb"] = outp.tile([128, C], f32,
                                                     tag="ybig", name="ybig")
                            py = ps_mm.tile([128, 512], f32, tag="mm",
                                            name="py")
                            for t in range(2):
                                nc.tensor.matmul(
                                    py[:], ot[t][:, bsl], pw[:, t, esl],
                                    start=(t == 0), stop=(t == 1))
                            nc.vector.tensor_copy(st["yb"][:, esl], py[:])
                            if ech == 1:
                                dma_eng.dma_start(y_d.ap()[bsl, :],
                                                  st["yb"][:])
                        return f
                    return [(mk(ech), 2 * MM_NS) for ech in range(2)]

                def emit_units(units):
                    for fn, cost in units:
                        fn()
                        clock["pe"] += cost

                # ---- upfront PE work: q0 c0/c1, k0 c0, V0 ----
                emit_units(qk_units(wq, qt[0], 0, 0))
                emit_units(qk_units(wq, qt[0], 0, 1))
                emit_units(qk_units(wk, kt[0], 0, 0))
                emit_units(v_units(0))

                # ---- background queue with deadlines (step index) ----
                for j in (1, 2, 3):
                    for u in qk_units(wk, kt[0], 0, j):
                        push(4 * j - 1, *u)
                for nb in range(1, NB):
                    for u in v_units(nb):
                        push(nb, *u)
                for j in (2, 3):
                    for u in qk_units(wq, qt[0], 0, j):
                        push(62, *u)
                for j, due in ((0, 30), (1, 30), (2, 94), (3, 94)):
                    for u in qk_units(wq, qt[1], 1, j):
                        push(due, *u)
                for j in range(4):
                    for u in qk_units(wk, kt[1], 1, j):
                        push(31 + 4 * j, *u)

                # ---- attention stream: 128 (nt, h, mb) steps ----
                # nt-major so nt0's projection becomes background work
                # halfway through, spreading its y stores.
                for nt in range(NTC):
                    qsl = slice(nt * NT, (nt + 1) * NT)
                    for h in range(HPC):
                        t, hi = divmod(h, 2)
                        psl = slice(hi * 64, (hi + 1) * 64)
                        po = ps_o.tile([HD + 1, NT], f32, tag="po", name="po")
                        for mb in range(MB):
                            step = nt * 64 + h * 16 + mb
                            drain(step)
                            msl = slice(mb * 128, (mb + 1) * 128)
                            psb = ps_s.tile([128, NT], f32, tag="ps",
                                            name="psb")
                            for sc in range(NT // 512):
                                ssl = slice(sc * 512, (sc + 1) * 512)
                                nc.tensor.matmul(
                                    psb[:, ssl],
                                    kt[t][psl, msl],
                                    qt[t][psl, nt * NT + sc * 512:
                                          nt * NT + (sc + 1) * 512],
                                    start=True, stop=True)
                            clock["pe"] += 2 * MM_NS
                            eb = ebp.tile([128, NT], mdt, tag="eb",
                                          name="eb")
                            nc.scalar.activation(
                                out=eb[:], in_=psb[:], func=Exp, scale=SCALE)
                            clock["act"] = max(clock["act"],
                                               clock["pe"]) + ACT_NS
                            drain(step, slack=True)
                            for sc in range(NT // 512):
                                ssl = slice(sc * 512, (sc + 1) * 512)
                                nc.tensor.matmul(
                                    po[:, ssl],
                                    vt4[:, mb, h, :],
                                    eb[:, ssl],
                                    start=(mb == 0), stop=(mb == MB - 1))
                            clock["pe"] += 2 * MM_NS
                        # normalize: O^T[dh, n] * (1/Z[n]); one copy frees po
                        poc = work.tile([HD + 1, NT], f32, tag="poc",
                                        name="poc", bufs=1)
                        nc.vector.tensor_copy(poc[:], po[:])
                        rz = work.tile([1, NT], f32, tag="rz", name="rz")
                        nc.vector.reciprocal(rz[:], poc[HD:HD + 1, :])
                        rzb = work.tile([64, NT], f32, tag="rzb", name="rzb")
                        nc.gpsimd.partition_broadcast(rzb[:], rz[:])
                        nc.vector.tensor_tensor(
                            ot[t][psl, qsl], poc[0:HD, :], rzb[:], mult)
                        # once the last head of a query tile is normalized,
                        # its projection becomes background work
                        if h == HPC - 1:
                            for nb in range(nt * NT // 128,
                                            (nt + 1) * NT // 128):
                                for u in proj_units(nt, nb):
                                    push(998 + nt, *u)
                drain(float("inf"), slack=True)

            if reps == 1:
                emit_body()
            else:
                with tc.For_i(0, reps, 1):
                    emit_body()

    nc.compile()
    return nc


def _get_nc(reps=1, phase="full", dtype="bf16", opts=None):
    key = f"nc{reps}-{phase}-{dtype}-{sorted((opts or {}).items())}"
    if key not in _state:
        _state[key] = _build_nc(reps, phase, dtype, opts)
    return _state[key]


def _shard_inputs(x, qkv_w, proj_w, dtype="bf16"):
    """Per-core input maps. Core c: batch c//4, heads 4*(c%4)..4*(c%4)+3."""
    if dtype == "bf16":
        import ml_dtypes
        cast = lambda a: np.ascontiguousarray(a).astype(ml_dtypes.bfloat16)
    else:
        cast = lambda a: np.ascontiguousarray(a, np.float32)
    in_maps = []
    for c in range(NCORES):
        b, g = divmod(c, TPG)
        dsl = slice(g * D, (g + 1) * D)
        in_maps.append({
            "xT": cast(x[b].T),
            "wqT": cast(qkv_w[dsl, :].T),
            "wkT": cast(qkv_w[C:2 * C][dsl, :].T),
            "wvT": cast(qkv_w[2 * C:][dsl, :].T),
            "pwT": cast(proj_w[:, dsl].T),
        })
    return in_maps


def _make_runner(nc, donate=True):
    """Jitted 8-core SPMD runner for a built Bass module."""
    import jax
    import concourse.mybir as mybir
    from concourse import bass2jax

    bass2jax.install_neuronx_cc_hook()

    partition_name = (nc.partition_id_tensor.name
                      if nc.partition_id_tensor else None)
    in_names, out_names, out_avals, zero_shapes = [], [], [], []
    for alloc in nc.m.functions[0].allocations:
        if not isinstance(alloc, mybir.MemoryLocationSet):
            continue
        name = alloc.memorylocations[0].name
        if alloc.kind == "ExternalInput":
            if name != partition_name:
                in_names.append(name)
        elif alloc.kind == "ExternalOutput":
            shape = tuple(alloc.tensor_shape)
            dtype = mybir.dt.np(alloc.dtype)
            out_names.append(name)
            out_avals.append(jax.core.ShapedArray(shape, dtype))
            zero_shapes.append((shape, dtype))
    n_params = len(in_names)
    all_in_names = list(in_names) + list(out_names)
    if partition_name is not None:
        all_in_names.append(partition_name)
    donate_idx = tuple(range(n_params, n_params + len(out_names))) if donate \
        else ()

    def _body(*args):
        operands = list(args)
        if partition_name is not None:
            operands.append(bass2jax.partition_id_tensor())
        outs = bass2jax._bass_exec_p.bind(
            *operands,
            out_avals=tuple(out_avals),
            in_names=tuple(all_in_names),
            out_names=tuple(out_names),
            lowering_input_output_aliases=(),
            sim_require_finite=True,
            sim_require_nnan=True,
            nc=nc,
        )
        return tuple(outs)

    devices = jax.devices()[:NCORES]
    mesh = bass2jax.Mesh(np.asarray(devices), ("core",))
    spec = (bass2jax.PartitionSpec("core"),)
    sharded = jax.jit(
        bass2jax.shard_map(
            _body, mesh=mesh,
            in_specs=spec * (n_params + len(out_names)),
            out_specs=spec * len(out_names),
            check_rep=False),
        donate_argnums=donate_idx, keep_unused=True)

    meta = dict(in_names=in_names, out_names=out_names, out_avals=out_avals,
                zero_shapes=zero_shapes, mesh=mesh)
    return sharded, meta


def _get_runner():
    if "runner" in _state:
        return _state["runner"]
    nc = _get_nc(1)
    sharded, meta = _make_runner(nc, donate=True)

    def run(in_maps):
        concat_in = [
            np.concatenate([np.asarray(m[name]) for m in in_maps], axis=0)
            for name in meta["in_names"]
        ]
        concat_zeros = [
            np.zeros((NCORES * s[0], *s[1:]), dt)
            for s, dt in meta["zero_shapes"]
        ]
        out_arrs = sharded(*concat_in, *concat_zeros)
        out_avals = meta["out_avals"]
        return [
            {name: np.asarray(out_arrs[i]).reshape(
                NCORES, *out_avals[i].shape)[c]
             for i, name in enumerate(meta["out_names"])}
            for c in range(NCORES)
        ]

    _state["runner"] = run
    return run


def _combine(results, proj_b):
    """Sum the 4 tensor-parallel partial projections per batch, add bias."""
    out = np.empty((B, N, C), np.float32)
    for b in range(B):
        acc = results[b * TPG + 0]["y"].astype(np.float32).copy()
        for g in range(1, TPG):
            acc += results[b * TPG + g]["y"]
        out[b] = acc + proj_b[None, :]
    return out


def kernel(x, qkv_w, proj_w, proj_b):
    x = np.asarray(x, np.float32)
    qkv_w = np.asarray(qkv_w, np.float32)
    proj_w = np.asarray(proj_w, np.float32)
    proj_b = np.asarray(proj_b, np.float32)
    run = _get_runner()
    results = run(_shard_inputs(x, qkv_w, proj_w))
    return _combine(results, proj_b)


def make_timing_fn(reps, in_maps, phase="full", dtype="bf16", opts=None):
    """Device-resident, non-donating executor of the reps-times kernel.

    Returns fn() that launches one execution and blocks until done. Inputs
    (and dummy zero outputs) are placed on device once, so repeated calls
    measure dispatch + on-device execution only.
    """
    import jax
    from jax.sharding import NamedSharding
    from concourse import bass2jax

    nc = _get_nc(reps, phase, dtype, opts)
    sharded, meta = _make_runner(nc, donate=False)
    shd = NamedSharding(meta["mesh"], bass2jax.PartitionSpec("core"))
    dev_in = [
        jax.device_put(
            np.concatenate([np.asarray(m[name]) for m in in_maps], axis=0),
            shd)
        for name in meta["in_names"]
    ]
    dev_zero = [
        jax.device_put(np.zeros((NCORES * s[0], *s[1:]), dt), shd)
        for s, dt in meta["zero_shapes"]
    ]

    def fn():
        outs = sharded(*dev_in, *dev_zero)
        for o in outs:
            o.block_until_ready()
        return outs

    return fn


# revision 14
# speedup vs baseline: 1.2695x; 1.0790x over previous
"""MemoryEfficientAttention on 8 TRN2 NeuronCores.

Full inputs in, full output out. Sharding: data-parallel over batch (2) x
tensor-parallel over heads (16 heads -> 4 heads/core). Each core computes
qkv projection for its heads, flash-style attention, and a partial output
projection over its 256 head-dims; the host sums the 4 partial projections
per batch and adds the bias.

V2: all operands bf16 (rel err ~7e-3 vs 2e-2 budget), f32 psum accumulate.
The kernel is emitted as a software-pipelined stream built around the Act
engine (exp is the densest fixed cost: 16.8M exps = ~110us at 128 lanes *
1.2GHz). A virtual-clock scheduler interleaves background PE work (qkv
chunk matmuls, V blocks, output-projection blocks) into the attention
S->exp->PV stream so exp starts ~10us in and never starves:
  upfront: q0c0,q0c1,k0c0,V0; then 128 steps of (S, exp, PV) per
  (head, query-tile, key-block) with deadline-driven drain of queued units.
Device layouts (T = contraction dim on partitions):
  xt  [128,8ko,2048]  x[b]^T          qt/kt [128,2048] per head-pair
  vt  [128,16nb,4h,65] V with a ones column (softmax Z via PV matmul)
  po  [65,1024] psum   S^T [128keys,1024q] psum -> exp -> eb bf16
  normalization: reciprocal + gpsimd partition_broadcast + DVE multiply.
DMA: inputs on sync queue (weights first, xt in 4 token-chunks so the
first matmuls start after ~3us); y stores on the gpsimd queue.
"""

import heapq
import numpy as np

B, N, C = 2, 2048, 1024
H, HD = 16, 64
NCORES = 8
TPG = 4              # tensor-parallel cores per batch
HPC = H // TPG       # 4 heads per core
D = HPC * HD         # 256 local head dims
KO = C // 128        # 8 contraction subtiles of the model dim
NB = N // 128        # 16 token blocks
MB = N // 128        # 16 key blocks
NT = 1024            # query-tile width in attention
NTC = N // NT
SCALE = HD ** -0.5

_state = {}


def _build_nc(reps=1, phase="full", dtype="bf16", opts=None):
    import concourse.bass as bass
    import concourse.tile as tile
    import concourse.mybir as mybir
    from concourse import bacc

    opts = {**dict(ps_bufs=2, mm_bufs=2, eb_bufs=4, outp_bufs=2,
                   po_bufs=1, act_ns=953, mm_ns=185, s_ns=207, pv_ns=210),
            **(opts or {})}
    f32 = mybir.dt.float32
    mdt = mybir.dt.bfloat16 if dtype == "bf16" else mybir.dt.float32r
    Exp = mybir.ActivationFunctionType.Exp
    mult = mybir.AluOpType.mult
    ACT_NS = opts["act_ns"]
    MM_NS = opts["mm_ns"]       # virtual cost of one N=512 matmul
    S_NS = opts["s_ns"]
    PV_NS = opts["pv_ns"]

    nc = bacc.Bacc("TRN2", target_bir_lowering=False, debug=False,
                   num_devices=NCORES)

    xT_d = nc.dram_tensor("xT", [C, N], mdt, kind="ExternalInput")
    wqT_d = nc.dram_tensor("wqT", [C, D], mdt, kind="ExternalInput")
    wkT_d = nc.dram_tensor("wkT", [C, D], mdt, kind="ExternalInput")
    wvT_d = nc.dram_tensor("wvT", [C, D], mdt, kind="ExternalInput")
    pwT_d = nc.dram_tensor("pwT", [D, C], mdt, kind="ExternalInput")
    y_d = nc.dram_tensor("y", [N, C], f32, kind="ExternalOutput")

    with tile.TileContext(nc) as tc:
        with (
            tc.tile_pool(name="big", bufs=1) as big,
            tc.tile_pool(name="work", bufs=2) as work,
            tc.tile_pool(name="ebp", bufs=opts["eb_bufs"]) as ebp,
            tc.tile_pool(name="outp", bufs=opts["outp_bufs"]) as outp,
            tc.tile_pool(name="ps_mm", bufs=opts["mm_bufs"], space="PSUM") as ps_mm,
            tc.tile_pool(name="ps_s", bufs=opts["ps_bufs"], space="PSUM") as ps_s,
            tc.tile_pool(name="ps_o", bufs=opts["po_bufs"], space="PSUM") as ps_o,
        ):
            xt = big.tile([128, KO, N], mdt, tag="xt")
            wq = big.tile([128, KO, D], mdt, tag="wq")
            wk = big.tile([128, KO, D], mdt, tag="wk")
            wv = big.tile([128, KO, D], mdt, tag="wv")
            pw = big.tile([128, D // 128, C], mdt, tag="pw")
            # per-head q with the OTHER head's 64 rows zeroed: S then
            # contracts over K=128 (zeros contribute nothing), which runs
            # ~2x faster per column than K=64 on HW
            qz = [[big.tile([128, N], mdt, tag=f"qz{t}{i}", name=f"qz{t}{i}")
                   for i in range(2)] for t in range(2)]
            kt = [big.tile([128, N], mdt, tag=f"kt{t}", name=f"kt{t}")
                  for t in range(2)]
            vt = big.tile([128, NB, HPC * (HD + 1)], mdt, tag="vt")
            ot = [big.tile([128, N], mdt, tag=f"ot{t}", name=f"ot{t}")
                  for t in range(2)]
            vt4 = vt[:].rearrange("p nb (h c) -> p nb h c", c=HD + 1)

            def emit_prolog():
                # zero halves persist across hardware-loop iterations
                for t in range(2):
                    nc.vector.memset(qz[t][0][64:128, :], 0.0)
                    nc.vector.memset(qz[t][1][0:64, :], 0.0)

            def emit_body():
                # ---- input DMA: weights first, xt in 4 token-chunks ----
                nc.sync.dma_start(
                    wq[:], wqT_d.ap().rearrange("(ko p) d -> p ko d", p=128))
                nc.sync.dma_start(
                    wk[:], wkT_d.ap().rearrange("(ko p) d -> p ko d", p=128))
                nc.sync.dma_start(
                    wv[:], wvT_d.ap().rearrange("(ko p) d -> p ko d", p=128))
                for ch in range(4):
                    s = slice(ch * 512, (ch + 1) * 512)
                    nc.sync.dma_start(
                        xt[:, :, s],
                        xT_d.ap()[:, s].rearrange("(ko p) n -> p ko n", p=128))
                nc.sync.dma_start(
                    pw[:], pwT_d.ap().rearrange("(t p) e -> p t e", p=128))
                # ones column of vt: memset f32 staging + DVE cast-copy
                ones_sb = work.tile([128, NB * HPC], f32, tag="ones_sb",
                                    name="ones_sb", bufs=1)
                nc.vector.memset(ones_sb[:], 1.0)
                nc.vector.tensor_copy(
                    vt4[:, :, :, HD:HD + 1],
                    ones_sb[:].rearrange("p (nb h) -> p nb h", nb=NB
                                         ).unsqueeze(-1))

                # ---- background unit machinery ----
                clock = {"pe": 0.0, "act": 0.0}
                Q = []
                ctr = [0]

                def push(due, fn, cost):
                    ctr[0] += 1
                    heapq.heappush(Q, (due, ctr[0], fn, cost))

                def drain(s, slack=False):
                    while Q and (Q[0][0] <= s
                                 or (slack and clock["pe"] < clock["act"])):
                        _, _, fn, cost = heapq.heappop(Q)
                        fn()
                        clock["pe"] += cost

                def chain(spec):
                    """One 8-deep psum accumulation chain (q/k/v block).

                    Returns (mm(j), finish, per-mm virtual cost)."""
                    kind = spec[0]
                    st = {}
                    if kind in ("q", "k"):
                        _, t, ch = spec
                        nsl = slice(ch * 512, (ch + 1) * 512)
                        dsl = slice(t * 128, (t + 1) * 128)
                        w = wq if kind == "q" else wk

                        def mm(j):
                            if j == 0:
                                st["pm"] = ps_mm.tile([128, 512], f32,
                                                      tag="mm", name="pm")
                            nc.tensor.matmul(
                                st["pm"][:], w[:, j, dsl], xt[:, j, nsl],
                                start=(j == 0), stop=(j == KO - 1))

                        def fin():
                            pm = st["pm"]
                            if kind == "k":
                                nc.vector.tensor_copy(kt[t][:, nsl], pm[:])
                            else:
                                nc.vector.tensor_copy(
                                    qz[t][0][0:64, nsl], pm[0:64, :])
                                nc.vector.tensor_copy(
                                    qz[t][1][64:128, nsl], pm[64:128, :])
                        return mm, fin, MM_NS
                    _, nb = spec
                    bsl = slice(nb * 128, (nb + 1) * 128)

                    def mm(j):
                        if j == 0:
                            st["pm"] = ps_mm.tile([128, 512], f32,
                                                  tag="mm", name="pm")
                        nc.tensor.matmul(
                            st["pm"][:, :D], xt[:, j, bsl], wv[:, j, :],
                            start=(j == 0), stop=(j == KO - 1))

                    def fin():
                        nc.vector.tensor_copy(
                            vt4[:, nb, :, 0:HD],
                            st["pm"][:, :D].rearrange("p (h c) -> p h c",
                                                      c=HD))
                    return mm, fin, MM_NS * 0.6

                def pair_units(*specs):
                    """Interleave 2 chains mm-by-mm so ldweights of one
                    hides behind the other's stream; 8 units of 1 mm each
                    per chain."""
                    chains = [chain(s) for s in specs]
                    cost = sum(c for _, _, c in chains)

                    def mk(j):
                        def f():
                            for mm, _, _ in chains:
                                mm(j)
                            if j == KO - 1:
                                for _, fin, _ in chains:
                                    fin()
                        return f
                    return [(mk(j), cost) for j in range(KO)]

                def proj_units(nt, nb):
                    """partial output projection of token block nb: 1 unit
                    with the two 512-col chains interleaved.

                    y stores: nt0 on the gpsimd queue (Act still busy with
                    exps then), nt1 on the Act queue (idle in the tail, and
                    keeps the last partition_broadcasts unblocked on Pool).
                    """
                    bsl = slice(nb * 128, (nb + 1) * 128)
                    dma_eng = nc.gpsimd if nt == 0 else nc.scalar

                    def f():
                        yb = outp.tile([128, C], f32, tag="ybig",
                                       name="ybig")
                        pa = ps_mm.tile([128, 512], f32, tag="mm", name="pa")
                        pb = ps_mm.tile([128, 512], f32, tag="mm", name="pb")
                        for t in range(2):
                            nc.tensor.matmul(
                                pa[:], ot[t][:, bsl], pw[:, t, 0:512],
                                start=(t == 0), stop=(t == 1))
                            nc.tensor.matmul(
                                pb[:], ot[t][:, bsl], pw[:, t, 512:1024],
                                start=(t == 0), stop=(t == 1))
                        nc.vector.tensor_copy(yb[:, 0:512], pa[:])
                        nc.vector.tensor_copy(yb[:, 512:1024], pb[:])
                        dma_eng.dma_start(y_d.ap()[bsl, :], yb[:])
                    return [(f, 4 * MM_NS)]

                def emit_units(units):
                    for fn, cost in units:
                        fn()
                        clock["pe"] += cost

                # ---- upfront PE work: q0 c0/c1, k0 c0 + V0 ----
                emit_units(pair_units(("q", 0, 0), ("q", 0, 1)))
                emit_units(pair_units(("k", 0, 0), ("v", 0)))

                # ---- background queue with deadlines (step index) ----
                pairs = [
                    (1, ("k", 0, 1), ("v", 1)),
                    (2, ("k", 0, 2), ("v", 2)),
                    (3, ("k", 0, 3), ("v", 3)),
                    (4, ("v", 4), ("v", 5)),
                    (6, ("v", 6), ("v", 7)),
                    (8, ("v", 8), ("v", 9)),
                    (10, ("v", 10), ("v", 11)),
                    (12, ("v", 12), ("v", 13)),
                    (14, ("v", 14), ("v", 15)),
                    (30, ("q", 1, 0), ("q", 1, 1)),
                    (31, ("k", 1, 0), ("k", 1, 1)),
                    (35, ("k", 1, 2), ("k", 1, 3)),
                    (62, ("q", 0, 2), ("q", 0, 3)),
                    (94, ("q", 1, 2), ("q", 1, 3)),
                ]
                for due, *specs in pairs:
                    for u in pair_units(*specs):
                        push(due, *u)

                # ---- attention stream: 128 (nt, h, mb) steps ----
                # nt-major so nt0's projection becomes background work
                # halfway through, spreading its y stores.
                for nt in range(NTC):
                    qsl = slice(nt * NT, (nt + 1) * NT)
                    for h in range(HPC):
                        t, hi = divmod(h, 2)
                        psl = slice(hi * 64, (hi + 1) * 64)
                        po = ps_o.tile([HD + 1, NT], f32, tag="po", name="po")
                        for mb in range(MB):
                            step = nt * 64 + h * 16 + mb
                            drain(step)
                            msl = slice(mb * 128, (mb + 1) * 128)
                            psb = ps_s.tile([128, NT], f32, tag="ps",
                                            name="psb")
                            for sc in range(NT // 512):
                                ssl = slice(sc * 512, (sc + 1) * 512)
                                nc.tensor.matmul(
                                    psb[:, ssl],
                                    kt[t][:, msl],
                                    qz[t][hi][:, nt * NT + sc * 512:
                                              nt * NT + (sc + 1) * 512],
                                    start=True, stop=True)
                            clock["pe"] += 2 * S_NS
                            eb = ebp.tile([128, NT], mdt, tag="eb",
                                          name="eb")
                            nc.scalar.activation(
                                out=eb[:], in_=psb[:], func=Exp, scale=SCALE)
                            clock["act"] = max(clock["act"],
                                               clock["pe"]) + ACT_NS
                            drain(step, slack=True)
                            for sc in range(NT // 512):
                                ssl = slice(sc * 512, (sc + 1) * 512)
                                nc.tensor.matmul(
                                    po[:, ssl],
                                    vt4[:, mb, h, :],
                                    eb[:, ssl],
                                    start=(mb == 0), stop=(mb == MB - 1))
                            clock["pe"] += 2 * PV_NS
                        # normalize: O^T[dh, n] * (1/Z[n]); one copy frees po
                        poc = work.tile([HD + 1, NT], f32, tag="poc",
                                        name="poc", bufs=1)
                        nc.vector.tensor_copy(poc[:], po[:])
                        rz = work.tile([1, NT], f32, tag="rz", name="rz")
                        nc.vector.reciprocal(rz[:], poc[HD:HD + 1, :])
                        rzb = work.tile([64, NT], f32, tag="rzb", name="rzb")
                        nc.gpsimd.partition_broadcast(rzb[:], rz[:])
                        nc.vector.tensor_tensor(
                            ot[t][psl, qsl], poc[0:HD, :], rzb[:], mult)
                        # once the last head of a query tile is normalized,
                        # its projection becomes background work
                        if h == HPC - 1:
                            for nb in range(nt * NT // 128,
                                            (nt + 1) * NT // 128):
                                for u in proj_units(nt, nb):
                                    push(998 + nt, *u)
                drain(float("inf"), slack=True)

            emit_prolog()
            if reps == 1:
                emit_body()
            else:
                with tc.For_i(0, reps, 1):
                    emit_body()

    nc.compile()
    return nc


def _get_nc(reps=1, phase="full", dtype="bf16", opts=None):
    key = f"nc{reps}-{phase}-{dtype}-{sorted((opts or {}).items())}"
    if key not in _state:
        _state[key] = _build_nc(reps, phase, dtype, opts)
    return _state[key]


def _shard_inputs(x, qkv_w, proj_w, dtype="bf16"):
    """Per-core input maps. Core c: batch c//4, heads 4*(c%4)..4*(c%4)+3."""
    if dtype == "bf16":
        import ml_dtypes
        cast = lambda a: np.ascontiguousarray(a).astype(ml_dtypes.bfloat16)
    else:
        cast = lambda a: np.ascontiguousarray(a, np.float32)
    in_maps = []
    for c in range(NCORES):
        b, g = divmod(c, TPG)
        dsl = slice(g * D, (g + 1) * D)
        in_maps.append({
            "xT": cast(x[b].T),
            "wqT": cast(qkv_w[dsl, :].T),
            "wkT": cast(qkv_w[C:2 * C][dsl, :].T),
            "wvT": cast(qkv_w[2 * C:][dsl, :].T),
            "pwT": cast(proj_w[:, dsl].T),
        })
    return in_maps


def _make_runner(nc, donate=True):
    """Jitted 8-core SPMD runner for a built Bass module."""
    import jax
    import concourse.mybir as mybir
    from concourse import bass2jax

    bass2jax.install_neuronx_cc_hook()

    partition_name = (nc.partition_id_tensor.name
                      if nc.partition_id_tensor else None)
    in_names, out_names, out_avals, zero_shapes = [], [], [], []
    for alloc in nc.m.functions[0].allocations:
        if not isinstance(alloc, mybir.MemoryLocationSet):
            continue
        name = alloc.memorylocations[0].name
        if alloc.kind == "ExternalInput":
            if name != partition_name:
                in_names.append(name)
        elif alloc.kind == "ExternalOutput":
            shape = tuple(alloc.tensor_shape)
            dtype = mybir.dt.np(alloc.dtype)
            out_names.append(name)
            out_avals.append(jax.core.ShapedArray(shape, dtype))
            zero_shapes.append((shape, dtype))
    n_params = len(in_names)
    all_in_names = list(in_names) + list(out_names)
    if partition_name is not None:
        all_in_names.append(partition_name)
    donate_idx = tuple(range(n_params, n_params + len(out_names))) if donate \
        else ()

    def _body(*args):
        operands = list(args)
        if partition_name is not None:
            operands.append(bass2jax.partition_id_tensor())
        outs = bass2jax._bass_exec_p.bind(
            *operands,
            out_avals=tuple(out_avals),
            in_names=tuple(all_in_names),
            out_names=tuple(out_names),
            lowering_input_output_aliases=(),
            sim_require_finite=True,
            sim_require_nnan=True,
            nc=nc,
        )
        return tuple(outs)

    devices = jax.devices()[:NCORES]
    mesh = bass2jax.Mesh(np.asarray(devices), ("core",))
    spec = (bass2jax.PartitionSpec("core"),)
    sharded = jax.jit(
        bass2jax.shard_map(
            _body, mesh=mesh,
            in_specs=spec * (n_params + len(out_names)),
            out_specs=spec * len(out_names),
            check_rep=False),
        donate_argnums=donate_idx, keep_unused=True)

    meta = dict(in_names=in_names, out_names=out_names, out_avals=out_avals,
                zero_shapes=zero_shapes, mesh=mesh)
    return sharded, meta


def _get_runner():
    if "runner" in _state:
        return _state["runner"]
    nc = _get_nc(1)
    sharded, meta = _make_runner(nc, donate=True)

    def run(in_maps):
        concat_in = [
            np.concatenate([np.asarray(m[name]) for m in in_maps], axis=0)
            for name in meta["in_names"]
        ]
        concat_zeros = [
            np.zeros((NCORES * s[0], *s[1:]), dt)
            for s, dt in meta["zero_shapes"]
        ]
        out_arrs = sharded(*concat_in, *concat_zeros)
        out_avals = meta["out_avals"]
        return [
            {name: np.asarray(out_arrs[i]).reshape(
                NCORES, *out_avals[i].shape)[c]
             for i, name in enumerate(meta["out_names"])}
            for c in range(NCORES)
        ]

    _state["runner"] = run
    return run


def _combine(results, proj_b):
    """Sum the 4 tensor-parallel partial projections per batch, add bias."""
    out = np.empty((B, N, C), np.float32)
    for b in range(B):
        acc = results[b * TPG + 0]["y"].astype(np.float32).copy()
        for g in range(1, TPG):
            acc += results[b * TPG + g]["y"]
        out[b] = acc + proj_b[None, :]
    return out


def kernel(x, qkv_w, proj_w, proj_b):
    x = np.asarray(x, np.float32)
    qkv_w = np.asarray(qkv_w, np.float32)
    proj_w = np.asarray(proj_w, np.float32)
    proj_b = np.asarray(proj_b, np.float32)
    run = _get_runner()
    results = run(_shard_inputs(x, qkv_w, proj_w))
    return _combine(results, proj_b)


def make_timing_fn(reps, in_maps, phase="full", dtype="bf16", opts=None):
    """Device-resident, non-donating executor of the reps-times kernel.

    Returns fn() that launches one execution and blocks until done. Inputs
    (and dummy zero outputs) are placed on device once, so repeated calls
    measure dispatch + on-device execution only.
    """
    import jax
    from jax.sharding import NamedSharding
    from concourse import bass2jax

    nc = _get_nc(reps, phase, dtype, opts)
    sharded, meta = _make_runner(nc, donate=False)
    shd = NamedSharding(meta["mesh"], bass2jax.PartitionSpec("core"))
    dev_in = [
        jax.device_put(
            np.concatenate([np.asarray(m[name]) for m in in_maps], axis=0),
            shd)
        for name in meta["in_names"]
    ]
    dev_zero = [
        jax.device_put(np.zeros((NCORES * s[0], *s[1:]), dt), shd)
        for s, dt in meta["zero_shapes"]
    ]

    def fn():
        outs = sharded(*dev_in, *dev_zero)
        for o in outs:
            o.block_until_ready()
        return outs

    return fn


# revision 26
# speedup vs baseline: 1.3252x; 1.0439x over previous
"""MemoryEfficientAttention on 8 TRN2 NeuronCores.

Full inputs in, full output out. Sharding: data-parallel over batch (2) x
tensor-parallel over heads (16 heads -> 4 heads/core). Each core computes
qkv projection for its heads, flash-style attention, and a partial output
projection over its 256 head-dims; the host sums the 4 partial projections
per batch and adds the bias.

V2: all operands bf16 (rel err ~7e-3 vs 2e-2 budget), f32 psum accumulate.
The kernel is emitted as a software-pipelined stream built around the Act
engine (exp is the densest fixed cost: 16.8M exps = ~110us at 128 lanes *
1.2GHz). A virtual-clock scheduler interleaves background PE work (qkv
chunk matmuls, V blocks, output-projection blocks) into the attention
S->exp->PV stream so exp starts ~10us in and never starves:
  upfront: q0c0,q0c1,k0c0,V0; then 128 steps of (S, exp, PV) per
  (head, query-tile, key-block) with deadline-driven drain of queued units.
Device layouts (T = contraction dim on partitions):
  xt  [128,8ko,2048]  x[b]^T          qt/kt [128,2048] per head-pair
  vt  [128,16nb,4h,65] V with a ones column (softmax Z via PV matmul)
  po  [65,1024] psum   S^T [128keys,1024q] psum -> exp -> eb bf16
  normalization: reciprocal + gpsimd partition_broadcast + DVE multiply.
DMA: inputs on sync queue (weights first, xt in 4 token-chunks so the
first matmuls start after ~3us); y stores on the gpsimd queue.
"""

import heapq
import numpy as np

B, N, C = 2, 2048, 1024
H, HD = 16, 64
NCORES = 8
TPG = 4              # tensor-parallel cores per batch
HPC = H // TPG       # 4 heads per core
D = HPC * HD         # 256 local head dims
KO = C // 128        # 8 contraction subtiles of the model dim
NB = N // 128        # 16 token blocks
MB = N // 128        # 16 key blocks
NT = 1024            # query-tile width in attention
NTC = N // NT
SCALE = HD ** -0.5

_state = {}


def _build_nc(reps=1, phase="full", dtype="bf16", opts=None):
    import concourse.bass as bass
    import concourse.tile as tile
    import concourse.mybir as mybir
    from concourse import bacc

    opts = {**dict(ps_bufs=2, mm_bufs=2, eb_bufs=4, outp_bufs=2,
                   po_bufs=1, act_ns=953, mm_ns=185, s_ns=207, pv_ns=210,
                   nobg=True),
            **(opts or {})}
    f32 = mybir.dt.float32
    mdt = mybir.dt.bfloat16 if dtype == "bf16" else mybir.dt.float32r
    Exp = mybir.ActivationFunctionType.Exp
    mult = mybir.AluOpType.mult
    ACT_NS = opts["act_ns"]
    MM_NS = opts["mm_ns"]       # virtual cost of one N=512 matmul
    S_NS = opts["s_ns"]
    PV_NS = opts["pv_ns"]

    nc = bacc.Bacc("TRN2", target_bir_lowering=False, debug=False,
                   num_devices=NCORES)

    xT_d = nc.dram_tensor("xT", [C, N], mdt, kind="ExternalInput")
    wqT_d = nc.dram_tensor("wqT", [C, D], mdt, kind="ExternalInput")
    wkT_d = nc.dram_tensor("wkT", [C, D], mdt, kind="ExternalInput")
    wvT_d = nc.dram_tensor("wvT", [C, D], mdt, kind="ExternalInput")
    pwT_d = nc.dram_tensor("pwT", [D, C], mdt, kind="ExternalInput")
    y_d = nc.dram_tensor("y", [N, C], f32, kind="ExternalOutput")

    with tile.TileContext(nc) as tc:
        with (
            tc.tile_pool(name="big", bufs=1) as big,
            tc.tile_pool(name="work", bufs=2) as work,
            tc.tile_pool(name="ebp", bufs=opts["eb_bufs"]) as ebp,
            tc.tile_pool(name="outp", bufs=opts["outp_bufs"]) as outp,
            tc.tile_pool(name="ps_mm", bufs=opts["mm_bufs"], space="PSUM") as ps_mm,
            tc.tile_pool(name="ps_s", bufs=opts["ps_bufs"], space="PSUM") as ps_s,
            tc.tile_pool(name="ps_o", bufs=opts["po_bufs"], space="PSUM") as ps_o,
        ):
            xt = big.tile([128, KO, N], mdt, tag="xt")
            wq = big.tile([128, KO, D], mdt, tag="wq")
            wk = big.tile([128, KO, D], mdt, tag="wk")
            wv = big.tile([128, KO, D], mdt, tag="wv")
            pw = big.tile([128, D // 128, C], mdt, tag="pw")
            # per-head q with the OTHER head's 64 rows zeroed: S then
            # contracts over K=128 (zeros contribute nothing), which runs
            # ~2x faster per column than K=64 on HW
            qz = [[big.tile([128, N], mdt, tag=f"qz{t}{i}", name=f"qz{t}{i}")
                   for i in range(2)] for t in range(2)]
            kt = [big.tile([128, N], mdt, tag=f"kt{t}", name=f"kt{t}")
                  for t in range(2)]
            vt = big.tile([128, NB, HPC * (HD + 1)], mdt, tag="vt")
            ot = [big.tile([128, N], mdt, tag=f"ot{t}", name=f"ot{t}")
                  for t in range(2)]
            vt4 = vt[:].rearrange("p nb (h c) -> p nb h c", c=HD + 1)

            def emit_prolog():
                # zero halves persist across hardware-loop iterations
                for t in range(2):
                    nc.vector.memset(qz[t][0][64:128, :], 0.0)
                    nc.vector.memset(qz[t][1][0:64, :], 0.0)
                if phase == "attn":
                    # attention-only timing variant: small finite operands
                    for t in range(2):
                        nc.vector.memset(qz[t][0][0:64, :], 0.01)
                        nc.vector.memset(qz[t][1][64:128, :], 0.01)
                        nc.vector.memset(kt[t][:], 0.01)
                    nc.vector.memset(vt[:], 0.01)

            def emit_body():
                # ---- input DMA: weights first, xt in 4 token-chunks ----
                nc.sync.dma_start(
                    wq[:], wqT_d.ap().rearrange("(ko p) d -> p ko d", p=128))
                nc.sync.dma_start(
                    wk[:], wkT_d.ap().rearrange("(ko p) d -> p ko d", p=128))
                nc.sync.dma_start(
                    wv[:], wvT_d.ap().rearrange("(ko p) d -> p ko d", p=128))
                for ch in range(4):
                    s = slice(ch * 512, (ch + 1) * 512)
                    nc.sync.dma_start(
                        xt[:, :, s],
                        xT_d.ap()[:, s].rearrange("(ko p) n -> p ko n", p=128))
                nc.sync.dma_start(
                    pw[:], pwT_d.ap().rearrange("(t p) e -> p t e", p=128))
                # ones column of vt: memset f32 staging + DVE cast-copy
                ones_sb = work.tile([128, NB * HPC], f32, tag="ones_sb",
                                    name="ones_sb", bufs=1)
                nc.vector.memset(ones_sb[:], 1.0)
                nc.vector.tensor_copy(
                    vt4[:, :, :, HD:HD + 1],
                    ones_sb[:].rearrange("p (nb h) -> p nb h", nb=NB
                                         ).unsqueeze(-1))

                # ---- background unit machinery ----
                clock = {"pe": 0.0, "act": 0.0}
                Q = []
                ctr = [0]

                def push(due, fn, cost):
                    ctr[0] += 1
                    heapq.heappush(Q, (due, ctr[0], fn, cost))

                def drain(s, slack=False, max_pull=10 ** 9):
                    pulled = 0
                    while Q and (Q[0][0] <= s
                                 or (slack and pulled < max_pull
                                     and clock["pe"] < clock["act"])):
                        _, _, fn, cost = heapq.heappop(Q)
                        fn()
                        clock["pe"] += cost
                        pulled += 1

                def chain(spec):
                    """One 8-deep psum accumulation chain (q/k/v block).

                    Returns (mm(j), finish, per-mm virtual cost)."""
                    kind = spec[0]
                    st = {}
                    if kind in ("q", "k"):
                        _, t, ch = spec
                        nsl = slice(ch * 512, (ch + 1) * 512)
                        dsl = slice(t * 128, (t + 1) * 128)
                        w = wq if kind == "q" else wk

                        def mm(j):
                            if j == 0:
                                st["pm"] = ps_mm.tile([128, 512], f32,
                                                      tag="mm", name="pm")
                            nc.tensor.matmul(
                                st["pm"][:], w[:, j, dsl], xt[:, j, nsl],
                                start=(j == 0), stop=(j == KO - 1))

                        def fin():
                            pm = st["pm"]
                            if kind == "k":
                                nc.vector.tensor_copy(kt[t][:, nsl], pm[:])
                            else:
                                nc.vector.tensor_copy(
                                    qz[t][0][0:64, nsl], pm[0:64, :])
                                nc.vector.tensor_copy(
                                    qz[t][1][64:128, nsl], pm[64:128, :])
                        return mm, fin, MM_NS
                    _, nb = spec
                    bsl = slice(nb * 128, (nb + 1) * 128)

                    def mm(j):
                        if j == 0:
                            st["pm"] = ps_mm.tile([128, 512], f32,
                                                  tag="mm", name="pm")
                        nc.tensor.matmul(
                            st["pm"][:, :D], xt[:, j, bsl], wv[:, j, :],
                            start=(j == 0), stop=(j == KO - 1))

                    def fin():
                        nc.vector.tensor_copy(
                            vt4[:, nb, :, 0:HD],
                            st["pm"][:, :D].rearrange("p (h c) -> p h c",
                                                      c=HD))
                    return mm, fin, MM_NS * 0.6

                def pair_units(*specs):
                    """Interleave 2 chains mm-by-mm so ldweights of one
                    hides behind the other's stream; 8 units of 1 mm each
                    per chain."""
                    chains = [chain(s) for s in specs]
                    cost = sum(c for _, _, c in chains)

                    def mk(j):
                        def f():
                            for mm, _, _ in chains:
                                mm(j)
                            if j == KO - 1:
                                for _, fin, _ in chains:
                                    fin()
                        return f
                    return [(mk(j), cost) for j in range(KO)]

                def proj_units(nt, nb):
                    """partial output projection of token block nb: 1 unit
                    with the two 512-col chains interleaved.

                    y stores: nt0 on the gpsimd queue (Act still busy with
                    exps then), nt1 on the Act queue (idle in the tail, and
                    keeps the last partition_broadcasts unblocked on Pool).
                    """
                    bsl = slice(nb * 128, (nb + 1) * 128)
                    dma_eng = nc.gpsimd if nt == 0 else nc.scalar

                    def f():
                        yb = outp.tile([128, C], f32, tag="ybig",
                                       name="ybig")
                        pa = ps_mm.tile([128, 512], f32, tag="mm", name="pa")
                        pb = ps_mm.tile([128, 512], f32, tag="mm", name="pb")
                        for t in range(2):
                            nc.tensor.matmul(
                                pa[:], ot[t][:, bsl], pw[:, t, 0:512],
                                start=(t == 0), stop=(t == 1))
                            nc.tensor.matmul(
                                pb[:], ot[t][:, bsl], pw[:, t, 512:1024],
                                start=(t == 0), stop=(t == 1))
                        nc.vector.tensor_copy(yb[:, 0:512], pa[:])
                        nc.vector.tensor_copy(yb[:, 512:1024], pb[:])
                        dma_eng.dma_start(y_d.ap()[bsl, :], yb[:])
                    return [(f, 4 * MM_NS)]

                def emit_units(units):
                    for fn, cost in units:
                        fn()
                        clock["pe"] += cost

                # ---- upfront PE work: q0 c0/c1, k0 c0 + V0 ----
                if phase != "attn":
                    emit_units(pair_units(("q", 0, 0), ("q", 0, 1)))
                    emit_units(pair_units(("k", 0, 0), ("v", 0)))

                # ---- background queue with deadlines (step index) ----
                pairs = [] if phase == "attn" else [
                    (1, ("k", 0, 1), ("v", 1)),
                    (2, ("k", 0, 2), ("v", 2)),
                    (3, ("k", 0, 3), ("v", 3)),
                    (4, ("v", 4), ("v", 5)),
                    (6, ("v", 6), ("v", 7)),
                    (8, ("v", 8), ("v", 9)),
                    (10, ("v", 10), ("v", 11)),
                    (12, ("v", 12), ("v", 13)),
                    (14, ("v", 14), ("v", 15)),
                    (30, ("q", 1, 0), ("q", 1, 1)),
                    (31, ("k", 1, 0), ("k", 1, 1)),
                    (35, ("k", 1, 2), ("k", 1, 3)),
                    (62, ("q", 0, 2), ("q", 0, 3)),
                    (94, ("q", 1, 2), ("q", 1, 3)),
                ]
                for due, *specs in pairs:
                    if opts.get("nobg"):
                        emit_units(pair_units(*specs))
                    else:
                        for u in pair_units(*specs):
                            push(due, *u)

                if phase == "qkv":
                    drain(float("inf"), slack=True)
                    yf = y_d.ap().rearrange("n c -> (n c)")
                    for i, tl in enumerate((kt[0], kt[1])):
                        sz = 128 * N // 2
                        nc.sync.dma_start(
                            yf[i * sz:(i + 1) * sz]
                            .rearrange("(p f) -> p f", p=128),
                            tl[:].bitcast(f32))
                    return

                # ---- attention stream: 128 (nt, h, mb) steps ----
                # nt-major so nt0's projection becomes background work
                # halfway through, spreading its y stores.
                for nt in range(NTC):
                    qsl = slice(nt * NT, (nt + 1) * NT)
                    for h in range(HPC):
                        t, hi = divmod(h, 2)
                        psl = slice(hi * 64, (hi + 1) * 64)
                        po = ps_o.tile([HD + 1, NT], f32, tag="po", name="po")
                        for mb in range(MB):
                            step = nt * 64 + h * 16 + mb
                            drain(step)
                            msl = slice(mb * 128, (mb + 1) * 128)
                            psb = ps_s.tile([128, NT], f32, tag="ps",
                                            name="psb")
                            for sc in range(NT // 512):
                                ssl = slice(sc * 512, (sc + 1) * 512)
                                nc.tensor.matmul(
                                    psb[:, ssl],
                                    kt[t][:, msl],
                                    qz[t][hi][:, nt * NT + sc * 512:
                                              nt * NT + (sc + 1) * 512],
                                    start=True, stop=True)
                            clock["pe"] += 2 * S_NS
                            eb = ebp.tile([128, NT], mdt, tag="eb",
                                          name="eb")
                            nc.scalar.activation(
                                out=eb[:], in_=psb[:], func=Exp, scale=SCALE)
                            clock["act"] = max(clock["act"],
                                               clock["pe"]) + ACT_NS
                            drain(step, slack=True, max_pull=1)
                            for sc in range(NT // 512):
                                ssl = slice(sc * 512, (sc + 1) * 512)
                                nc.tensor.matmul(
                                    po[:, ssl],
                                    vt4[:, mb, h, :],
                                    eb[:, ssl],
                                    start=(mb == 0), stop=(mb == MB - 1))
                            clock["pe"] += 2 * PV_NS
                        # normalize: O^T[dh, n] * (1/Z[n]); one copy frees po
                        poc = work.tile([HD + 1, NT], f32, tag="poc",
                                        name="poc", bufs=1)
                        nc.vector.tensor_copy(poc[:], po[:])
                        rz = work.tile([1, NT], f32, tag="rz", name="rz")
                        nc.vector.reciprocal(rz[:], poc[HD:HD + 1, :])
                        rzb = work.tile([64, NT], f32, tag="rzb", name="rzb")
                        nc.gpsimd.partition_broadcast(rzb[:], rz[:])
                        nc.vector.tensor_tensor(
                            ot[t][psl, qsl], poc[0:HD, :], rzb[:], mult)
                        # once the last head of a query tile is normalized,
                        # its projection becomes background work
                        if h == HPC - 1 and phase == "full":
                            for nb in range(nt * NT // 128,
                                            (nt + 1) * NT // 128):
                                for u in proj_units(nt, nb):
                                    push(998 + nt, *u)
                drain(float("inf"), slack=True)
                if phase in ("attn", "noproj"):
                    yf = y_d.ap().rearrange("n c -> (n c)")
                    for i, tl in enumerate(ot):
                        sz = 128 * N // 2
                        nc.sync.dma_start(
                            yf[i * sz:(i + 1) * sz]
                            .rearrange("(p f) -> p f", p=128),
                            tl[:].bitcast(f32))

            emit_prolog()
            if reps == 1:
                emit_body()
            else:
                with tc.For_i(0, reps, 1):
                    emit_body()

    nc.compile()
    return nc


def _get_nc(reps=1, phase="full", dtype="bf16", opts=None):
    key = f"nc{reps}-{phase}-{dtype}-{sorted((opts or {}).items())}"
    if key not in _state:
        _state[key] = _build_nc(reps, phase, dtype, opts)
    return _state[key]


def _shard_inputs(x, qkv_w, proj_w, dtype="bf16"):
    """Per-core input maps. Core c: batch c//4, heads 4*(c%4)..4*(c%4)+3."""
    if dtype == "bf16":
        import ml_dtypes
        cast = lambda a: np.ascontiguousarray(a).astype(ml_dtypes.bfloat16)
    else:
        cast = lambda a: np.ascontiguousarray(a, np.float32)
    in_maps = []
    for c in range(NCORES):
        b, g = divmod(c, TPG)
        dsl = slice(g * D, (g + 1) * D)
        in_maps.append({
            "xT": cast(x[b].T),
            "wqT": cast(qkv_w[dsl, :].T),
            "wkT": cast(qkv_w[C:2 * C][dsl, :].T),
            "wvT": cast(qkv_w[2 * C:][dsl, :].T),
            "pwT": cast(proj_w[:, dsl].T),
        })
    return in_maps


def _make_runner(nc, donate=True):
    """Jitted 8-core SPMD runner for a built Bass module."""
    import jax
    import concourse.mybir as mybir
    from concourse import bass2jax

    bass2jax.install_neuronx_cc_hook()

    partition_name = (nc.partition_id_tensor.name
                      if nc.partition_id_tensor else None)
    in_names, out_names, out_avals, zero_shapes = [], [], [], []
    for alloc in nc.m.functions[0].allocations:
        if not isinstance(alloc, mybir.MemoryLocationSet):
            continue
        name = alloc.memorylocations[0].name
        if alloc.kind == "ExternalInput":
            if name != partition_name:
                in_names.append(name)
        elif alloc.kind == "ExternalOutput":
            shape = tuple(alloc.tensor_shape)
            dtype = mybir.dt.np(alloc.dtype)
            out_names.append(name)
            out_avals.append(jax.core.ShapedArray(shape, dtype))
            zero_shapes.append((shape, dtype))
    n_params = len(in_names)
    all_in_names = list(in_names) + list(out_names)
    if partition_name is not None:
        all_in_names.append(partition_name)
    donate_idx = tuple(range(n_params, n_params + len(out_names))) if donate \
        else ()

    def _body(*args):
        operands = list(args)
        if partition_name is not None:
            operands.append(bass2jax.partition_id_tensor())
        outs = bass2jax._bass_exec_p.bind(
            *operands,
            out_avals=tuple(out_avals),
            in_names=tuple(all_in_names),
            out_names=tuple(out_names),
            lowering_input_output_aliases=(),
            sim_require_finite=True,
            sim_require_nnan=True,
            nc=nc,
        )
        return tuple(outs)

    devices = jax.devices()[:NCORES]
    mesh = bass2jax.Mesh(np.asarray(devices), ("core",))
    spec = (bass2jax.PartitionSpec("core"),)
    sharded = jax.jit(
        bass2jax.shard_map(
            _body, mesh=mesh,
            in_specs=spec * (n_params + len(out_names)),
            out_specs=spec * len(out_names),
            check_rep=False),
        donate_argnums=donate_idx, keep_unused=True)

    meta = dict(in_names=in_names, out_names=out_names, out_avals=out_avals,
                zero_shapes=zero_shapes, mesh=mesh)
    return sharded, meta


def _get_runner():
    if "runner" in _state:
        return _state["runner"]
    nc = _get_nc(1)
    sharded, meta = _make_runner(nc, donate=True)

    def run(in_maps):
        concat_in = [
            np.concatenate([np.asarray(m[name]) for m in in_maps], axis=0)
            for name in meta["in_names"]
        ]
        concat_zeros = [
            np.zeros((NCORES * s[0], *s[1:]), dt)
            for s, dt in meta["zero_shapes"]
        ]
        out_arrs = sharded(*concat_in, *concat_zeros)
        out_avals = meta["out_avals"]
        return [
            {name: np.asarray(out_arrs[i]).reshape(
                NCORES, *out_avals[i].shape)[c]
             for i, name in enumerate(meta["out_names"])}
            for c in range(NCORES)
        ]

    _state["runner"] = run
    return run


def _combine(results, proj_b):
    """Sum the 4 tensor-parallel partial projections per batch, add bias."""
    out = np.empty((B, N, C), np.float32)
    for b in range(B):
        acc = results[b * TPG + 0]["y"].astype(np.float32).copy()
        for g in range(1, TPG):
            acc += results[b * TPG + g]["y"]
        out[b] = acc + proj_b[None, :]
    return out


def kernel(x, qkv_w, proj_w, proj_b):
    x = np.asarray(x, np.float32)
    qkv_w = np.asarray(qkv_w, np.float32)
    proj_w = np.asarray(proj_w, np.float32)
    proj_b = np.asarray(proj_b, np.float32)
    run = _get_runner()
    results = run(_shard_inputs(x, qkv_w, proj_w))
    return _combine(results, proj_b)


def make_timing_fn(reps, in_maps, phase="full", dtype="bf16", opts=None):
    """Device-resident, non-donating executor of the reps-times kernel.

    Returns fn() that launches one execution and blocks until done. Inputs
    (and dummy zero outputs) are placed on device once, so repeated calls
    measure dispatch + on-device execution only.
    """
    import jax
    from jax.sharding import NamedSharding
    from concourse import bass2jax

    nc = _get_nc(reps, phase, dtype, opts)
    sharded, meta = _make_runner(nc, donate=False)
    shd = NamedSharding(meta["mesh"], bass2jax.PartitionSpec("core"))
    dev_in = [
        jax.device_put(
            np.concatenate([np.asarray(m[name]) for m in in_maps], axis=0),
            shd)
        for name in meta["in_names"]
    ]
    dev_zero = [
        jax.device_put(np.zeros((NCORES * s[0], *s[1:]), dt), shd)
        for s, dt in meta["zero_shapes"]
    ]

    def fn():
        outs = sharded(*dev_in, *dev_zero)
        for o in outs:
            o.block_until_ready()
        return outs

    return fn
